# revision 1
# baseline (speedup 1.0000x reference)
"""Fused transformer block (LN1 -> causal MHA -> residual -> LN2 -> FFN -> residual)
for Trainium2, distributed over 8 NeuronCores: tensor-parallel attention heads
within each batch (4 cores/batch, 4 heads/core), ReduceScatter of the row-split
Wo partial sums, then sequence-sharded FFN (512 tokens/core, full W1/W2).

Core c: batch g=c//4, rank r=c%4, heads 4r..4r+3.
- LN1 over the full 2048 tokens (replicated within the group), h transposed to
  feature-major; Q/K/V projected for this core's 4 heads only (col-split).
- Causal attention with block skipping: q-chunk i in {0..3} (512 queries)
  attends key blocks 0..4(i+1)-1 only; the 4 diagonal blocks get a triangular
  additive mask. exp via scalar engine; AV accumulates [64+1, 512] PSUM with a
  ones-row producing the softmax denominator.
- Wo row-split -> partial y [2048, 1024]; bf16 ReduceScatter over the 4-core
  group gives each rank its own 512-token slice of sum(y).
- x2 = x_q(f32) + rs (+bo); LN2 (512 tokens); FFN1/FFN2 with full weights
  (W1 resident in SBUF, W2 streamed once); out = x2 + ffn (+b2).
Matmuls in bf16 with fp32 PSUM; LN/softmax math fp32.
"""

import sys

import numpy as np

if "/opt/trn_rl_repo" not in sys.path:
    sys.path.insert(0, "/opt/trn_rl_repo")

import ml_dtypes

B, T, D = 2, 2048, 1024
H, HS = 16, 64
F = 4 * D
NCORES = 8
NH = 4            # heads per core
NF = NH * HS      # 256 features per core
TQ = T // 4       # 512 tokens per core after ReduceScatter
EPS = 1e-5
NEG = -1e9
GROUPS = [[0, 1, 2, 3], [4, 5, 6, 7]]

BF16 = ml_dtypes.bfloat16

_CACHE = {}


def _build(flags):
    """Build the Bass program (identical for all cores). flags: (has_bo, has_b2)."""
    import concourse.bass as bass
    import concourse.mybir as mybir
    import concourse.tile as tile
    from concourse import bacc
    from concourse.bass import ts
    from concourse.masks import make_identity

    has_bo, has_b2 = flags
    f32 = mybir.dt.float32
    bf16 = mybir.dt.bfloat16
    Alu = mybir.AluOpType
    Act = mybir.ActivationFunctionType

    nc = bacc.Bacc("TRN2", target_bir_lowering=False, debug=False, num_devices=8)

    # ---- DRAM I/O ----
    x_full = nc.dram_tensor("x_full", [T, D], bf16, kind="ExternalInput").ap()
    x_q = nc.dram_tensor("x_q", [TQ, D], f32, kind="ExternalInput").ap()
    maskD = nc.dram_tensor("maskD", [TQ, 2 * TQ], bf16, kind="ExternalInput").ap()
    wq = nc.dram_tensor("wq", [D, NF], bf16, kind="ExternalInput").ap()
    wk = nc.dram_tensor("wk", [D, NF], bf16, kind="ExternalInput").ap()
    wv = nc.dram_tensor("wv", [D, NF], bf16, kind="ExternalInput").ap()
    wo = nc.dram_tensor("wo", [D, D], bf16, kind="ExternalInput").ap()
    w1 = nc.dram_tensor("w1", [D, F], bf16, kind="ExternalInput").ap()
    w2 = nc.dram_tensor("w2", [F, D], bf16, kind="ExternalInput").ap()
    b1d = nc.dram_tensor("b1", [F], f32, kind="ExternalInput").ap()
    bod = nc.dram_tensor("bo", [D], f32, kind="ExternalInput").ap() if has_bo else None
    b2d = nc.dram_tensor("b2", [D], f32, kind="ExternalInput").ap() if has_b2 else None
    out = nc.dram_tensor("out", [TQ, D], f32, kind="ExternalOutput").ap()

    ag_in = [nc.dram_tensor(f"ag_in{kq}", [128, T], bf16, kind="Internal").ap()
             for kq in range(2)]
    ag_out = [nc.dram_tensor(f"ag_out{kq}", [4, 128, T], bf16,
                             kind="Internal").ap()
              for kq in range(2)]

    KT = T // 128      # 16 token tiles
    DC = D // 128      # 8 feature chunks of the model dim
    FC = F // 128      # 32 hidden chunks
    QS = TQ // 128     # 4 token tiles per 512-chunk

    with tile.TileContext(nc) as tc:
        with (
            tc.tile_pool(name="const", bufs=1) as cst,
            tc.tile_pool(name="w1P", bufs=1) as w1P,
            tc.tile_pool(name="actB", bufs=1) as actB,
        ):
            # --- constants ---
            ident = cst.tile([128, 128], bf16)
            make_identity(nc, ident)
            eps_t = cst.tile([128, 1], f32)
            nc.vector.memset(eps_t, EPS)
            zero_t = cst.tile([128, 1], f32)
            nc.vector.memset(zero_t, 0.0)
            b1_sb = cst.tile([128, FC], f32)
            nc.scalar.dma_start(out=b1_sb, in_=b1d.rearrange("(m p) -> p m", p=128))
            if has_bo:
                bo_b = cst.tile([128, D], f32)
                nc.scalar.dma_start(
                    out=bo_b,
                    in_=bass.AP(tensor=bod.tensor, offset=bod.offset,
                                ap=[[0, 128]] + list(bod.ap)))
            if has_b2:
                b2_b = cst.tile([128, D], f32)
                nc.scalar.dma_start(
                    out=b2_b,
                    in_=bass.AP(tensor=b2d.tensor, offset=b2d.offset,
                                ap=[[0, 128]] + list(b2d.ap)))
            mask_sb = cst.tile([128, QS, 2 * TQ], bf16)
            nc.scalar.dma_start(
                out=mask_sb, in_=maskD.rearrange("(k p) q -> p k q", p=128))

            # --- W1 resident in SBUF (64KB/partition), streamed early ---
            w1_sb = w1P.tile([128, DC, F], bf16)
            for k in range(DC):
                nc.scalar.dma_start(
                    out=w1_sb[:, k, :], in_=w1[ts(k, 128), :])
            # Wo, rows permuted to the gathered feature order: [128, 8, D]
            wo_sb = cst.tile([128, DC, D], bf16)
            nc.scalar.dma_start(
                out=wo_sb, in_=wo.rearrange("(j p) n -> p j n", p=128))

            # --- persistent activations (later-phase tiles created after
            # the h_fm pool closes to keep phase-1 SBUF pressure down) ---
            q_fm = [actB.tile([128, T], bf16, name=f"qfm{m}") for m in range(2)]
            k_fm = [actB.tile([128, T], bf16, name=f"kfm{m}") for m in range(2)]
            v_sb = [actB.tile([128, NH, HS + 1], bf16, name=f"vsb{t}")
                    for t in range(KT)]
            attnT = [actB.tile([128, T], bf16, name=f"at{d}") for d in range(2)]

            # ============ Phase 1+2: LN1 + transpose + QKV ============
            with tc.tile_pool(name="hfmP", bufs=1) as hfmP:
              h_fm = [hfmP.tile([128, T], bf16, name=f"hfm{d}")
                      for d in range(DC)]
              with (
                tc.tile_pool(name="ph12", bufs=3) as ph12,
                tc.tile_pool(name="wP", bufs=1) as wP,
                tc.tile_pool(name="psT", bufs=3, space="PSUM") as psT,
                tc.tile_pool(name="psV", bufs=2, space="PSUM") as psV,
                tc.tile_pool(name="psKQ", bufs=3, space="PSUM") as psKQ,
              ):
                wvc = wP.tile([128, DC, NF], bf16, name="wvc")
                nc.scalar.dma_start(
                    out=wvc, in_=wv.rearrange("(k p) c -> p k c", p=128))
                wkc = wP.tile([128, DC, NF], bf16, name="wkc")
                nc.scalar.dma_start(
                    out=wkc, in_=wk.rearrange("(k p) c -> p k c", p=128))
                wqc = wP.tile([128, DC, NF], bf16, name="wqc")
                nc.scalar.dma_start(
                    out=wqc, in_=wq.rearrange("(k p) c -> p k c", p=128))
                for t in range(KT):
                    nc.gpsimd.memset(v_sb[t][:, :, HS:HS + 1], 1.0)
                for t in range(KT):
                    xt = ph12.tile([128, D], bf16, tag="xt", name="xt")
                    nc.sync.dma_start(out=xt, in_=x_full[ts(t, 128), :])
                    xg = xt.rearrange("p (n f) -> p n f", f=512)
                    stats = ph12.tile([128, 2, 6], f32, tag="st", name="st")
                    for sg in range(2):
                        nc.vector.bn_stats(out=stats[:, sg, :], in_=xg[:, sg, :])
                    mv = ph12.tile([128, 2], f32, tag="mv", name="mv")
                    nc.vector.bn_aggr(out=mv, in_=stats)
                    rstd = ph12.tile([128, 1], f32, tag="rs", name="rs")
                    nc.scalar.activation(out=rstd, in_=mv[:, 1:2], func=Act.Sqrt,
                                         bias=eps_t, scale=1.0)
                    nc.vector.reciprocal(out=rstd, in_=rstd)
                    ht = ph12.tile([128, D], bf16, tag="ht", name="ht")
                    nc.vector.tensor_scalar(
                        out=ht, in0=xt, scalar1=mv[:, 0:1], scalar2=rstd,
                        op0=Alu.subtract, op1=Alu.mult)
                    for d in range(DC):
                        ps = psT.tile([128, 128], bf16, tag="tr", name="tr")
                        nc.tensor.transpose(ps, ht[:, ts(d, 128)], ident)
                        if d % 2 == 0:
                            nc.scalar.copy(out=h_fm[d][:, ts(t, 128)], in_=ps)
                        else:
                            nc.vector.tensor_copy(out=h_fm[d][:, ts(t, 128)],
                                                  in_=ps)
                    # V projection for this token tile
                    pv = psV.tile([128, NF], f32, tag="pv", name="pv")
                    for k in range(DC):
                        nc.tensor.matmul(pv, h_fm[k][:, ts(t, 128)],
                                         wvc[:, k, :],
                                         start=(k == 0), stop=(k == DC - 1))
                    nc.vector.tensor_copy(
                        out=v_sb[t][:, :, 0:HS],
                        in_=pv.rearrange("p (h d) -> p h d", d=HS))
                    if t % 4 == 3:
                        # K and Q projections for token chunk n (512 tokens)
                        n = t // 4
                        for fc in range(2):
                            psk = psKQ.tile([128, TQ], f32, tag="kq", name="psk")
                            for k in range(DC):
                                nc.tensor.matmul(psk, wkc[:, k, ts(fc, 128)],
                                                 h_fm[k][:, ts(n, 512)],
                                                 start=(k == 0),
                                                 stop=(k == DC - 1))
                            if fc == 0:
                                nc.vector.tensor_copy(
                                    out=k_fm[fc][:, ts(n, 512)], in_=psk)
                            else:
                                nc.scalar.copy(
                                    out=k_fm[fc][:, ts(n, 512)], in_=psk)
                            psq = psKQ.tile([128, TQ], f32, tag="kq", name="psq")
                            for k in range(DC):
                                nc.tensor.matmul(psq, wqc[:, k, ts(fc, 128)],
                                                 h_fm[k][:, ts(n, 512)],
                                                 start=(k == 0),
                                                 stop=(k == DC - 1))
                            if fc == 0:
                                nc.scalar.copy(
                                    out=q_fm[fc][:, ts(n, 512)], in_=psq)
                            else:
                                nc.vector.tensor_copy(
                                    out=q_fm[fc][:, ts(n, 512)], in_=psq)

            with tc.tile_pool(name="actB2", bufs=1) as actB2:
                z_sb = [actB2.tile([128, TQ], bf16, name=f"z{j}")
                        for j in range(DC)]
                x2_sb = [actB2.tile([128, D], f32, name=f"x2{i}")
                         for i in range(QS)]
                h2_fm = [actB2.tile([128, TQ], bf16, name=f"h2f{d}")
                         for d in range(DC)]

                # ================= Phase 3: attention =======================
                with (
                    tc.tile_pool(name="ph4", bufs=8) as ph4,
                    tc.tile_pool(name="smm", bufs=3) as smm,
                    tc.tile_pool(name="psS", bufs=2, space="PSUM") as psS,
                    tc.tile_pool(name="psAV", bufs=4, space="PSUM") as psAV,
                ):
                    LAG = 6
                    rq512 = (nc.sync.partition_id() % 4) * TQ

                    def emit_av(ent):
                        kq, pav0, pav1, pk, first, last, pe, qi = ent
                        for sub, pav in ((0, pav0), (1, pav1)):
                            nc.tensor.matmul(pav, v_sb[pk][:, 2 * kq + sub, :],
                                             pe[:, ts(sub, TQ)],
                                             start=first, stop=last)
                        if last:
                            for sub, pav in ((0, pav0), (1, pav1)):
                                raw = smm.tile([HS + 1, TQ], f32, tag="raw",
                                               name="raw")
                                nc.vector.tensor_copy(out=raw, in_=pav)
                                # [1,512] reciprocal as exp(-ln(x)) on the
                                # scalar engine (a single-partition DVE
                                # reciprocal costs 3.3us; Reciprocal act is
                                # blocked for accuracy)
                                lns = smm.tile([1, TQ], f32, tag="lns",
                                               name="lns")
                                nc.scalar.activation(
                                    out=lns, in_=raw[HS:HS + 1, :],
                                    func=Act.Ln, bias=zero_t[0:1, :],
                                    scale=1.0)
                                recip = smm.tile([1, TQ], f32, tag="recip",
                                                 name="recip")
                                nc.scalar.activation(
                                    out=recip, in_=lns,
                                    func=Act.Exp, bias=zero_t[0:1, :],
                                    scale=-1.0)
                                bcast = smm.tile([HS, TQ], f32, tag="bcast",
                                                 name="bcast")
                                nc.gpsimd.partition_broadcast(bcast, recip)
                                nc.vector.tensor_tensor(
                                    out=attnT[kq][ts(sub, HS), ts(qi, TQ)],
                                    in0=raw[0:HS, :], in1=bcast, op=Alu.mult)

                    def issue_ag(kq):
                        nc.sync.dma_start(out=ag_in[kq], in_=attnT[kq])
                        nc.gpsimd.collective_compute(
                            "AllGather",
                            mybir.AluOpType.bypass,
                            replica_groups=GROUPS,
                            ins=[ag_in[kq]],
                            outs=[ag_out[kq]],
                        )
                        # rank-sliced loads of the gathered blocks (own 512 toks)
                        for s in range(4):
                            src = bass.AP(
                                tensor=ag_out[kq].tensor,
                                offset=s * 128 * T + rq512,
                                ap=[[T, 128], [1, TQ]])
                            nc.sync.dma_start(out=z_sb[kq * 4 + s], in_=src)

                    pending = []
                    for kq in range(2):
                        for i in range(QS):
                            pav0 = psAV.tile([HS + 1, TQ], f32, tag="av",
                                             name="pav0")
                            pav1 = psAV.tile([HS + 1, TQ], f32, tag="av",
                                             name="pav1")
                            nkb = 4 * (i + 1)
                            for kb in range(nkb):
                                pss = psS.tile([128, 2 * TQ], f32, tag="s",
                                               name="pss")
                                for sub in range(2):
                                    ro = sub * HS
                                    nc.tensor.matmul(
                                        pss[:, ts(sub, TQ)],
                                        k_fm[kq][ro:ro + HS, ts(kb, 128)],
                                        q_fm[kq][ro:ro + HS, ts(i, TQ)],
                                        start=True, stop=True)
                                et = ph4.tile([128, 2 * TQ], bf16, tag="exp",
                                              name="et")
                                nc.scalar.activation(
                                    out=et, in_=pss, func=Act.Exp, scale=0.125,
                                    bias=zero_t)
                                if kb >= 4 * i:
                                    # zero the upper-triangle of the diagonal
                                    # block via a binary bf16 mask (SBUF-only,
                                    # so it can run on the idle gpsimd engine)
                                    nc.gpsimd.tensor_tensor(
                                        out=et, in0=et,
                                        in1=mask_sb[:, kb - 4 * i, :],
                                        op=Alu.mult)
                                pending.append(
                                    (kq, pav0, pav1, kb, kb == 0, kb == nkb - 1,
                                     et, i))
                                if len(pending) > LAG:
                                    ent = pending.pop(0)
                                    emit_av(ent)
                                    if ent[0] == 0 and ent[7] == QS - 1 and ent[5]:
                                        issue_ag(0)
                    for ent in pending:
                        emit_av(ent)
                    issue_ag(1)

                # ====== Phase 4: Wo (own tokens, gathered features) + x2 ====
                with (
                    tc.tile_pool(name="ph6", bufs=3) as ph6,
                    tc.tile_pool(name="psO", bufs=4, space="PSUM") as psO,
                    tc.tile_pool(name="psT2", bufs=4, space="PSUM") as psT2,
                ):
                    xq_sb = [ph6.tile([128, D], f32, tag=f"xq{i}", name=f"xq{i}")
                             for i in range(QS)]
                    for i in range(QS):
                        nc.sync.dma_start(out=xq_sb[i], in_=x_q[ts(i, 128), :])
                    # pass 1: first 4 gathered feature blocks (available after AG#0)
                    for i in range(QS):
                        pso = [psO.tile([128, 512], f32, tag="o", name="pso")
                               for n in range(2)]
                        for j in range(4):
                            for n in range(2):
                                nc.tensor.matmul(pso[n], z_sb[j][:, ts(i, 128)],
                                                 wo_sb[:, j, ts(n, 512)],
                                                 start=(j == 0), stop=(j == 3))
                        for n in range(2):
                            nc.vector.tensor_tensor(
                                out=x2_sb[i][:, ts(n, 512)], in0=pso[n],
                                in1=xq_sb[i][:, ts(n, 512)], op=Alu.add)
                    # pass 2: remaining 4 blocks (after AG#1), then LN2 per tile
                    for i in range(QS):
                        pso = [psO.tile([128, 512], f32, tag="o", name="pso")
                               for n in range(2)]
                        for j in range(4, DC):
                            for n in range(2):
                                nc.tensor.matmul(pso[n], z_sb[j][:, ts(i, 128)],
                                                 wo_sb[:, j, ts(n, 512)],
                                                 start=(j == 4), stop=(j == DC - 1))
                        for n in range(2):
                            nc.vector.tensor_tensor(
                                out=x2_sb[i][:, ts(n, 512)],
                                in0=x2_sb[i][:, ts(n, 512)],
                                in1=pso[n], op=Alu.add)
                        if has_bo:
                            nc.vector.tensor_tensor(
                                out=x2_sb[i], in0=x2_sb[i], in1=bo_b, op=Alu.add)
                        xg = x2_sb[i].rearrange("p (n f) -> p n f", f=512)
                        stats = ph6.tile([128, 2, 6], f32, tag="st", name="st6")
                        for sg in range(2):
                            nc.vector.bn_stats(out=stats[:, sg, :], in_=xg[:, sg, :])
                        mv = ph6.tile([128, 2], f32, tag="mv", name="mv6")
                        nc.vector.bn_aggr(out=mv, in_=stats)
                        rstd = ph6.tile([128, 1], f32, tag="rs", name="rs6")
                        nc.scalar.activation(out=rstd, in_=mv[:, 1:2], func=Act.Sqrt,
                                             bias=eps_t, scale=1.0)
                        nc.vector.reciprocal(out=rstd, in_=rstd)
                        h2t = ph6.tile([128, D], bf16, tag="h2t", name="h2t")
                        nc.vector.tensor_scalar(
                            out=h2t, in0=x2_sb[i], scalar1=mv[:, 0:1], scalar2=rstd,
                            op0=Alu.subtract, op1=Alu.mult)
                        for d in range(DC):
                            ps = psT2.tile([128, 128], bf16, tag="tr", name="tr2")
                            nc.tensor.transpose(ps, h2t[:, ts(d, 128)], ident)
                            if d % 2 == 0:
                                nc.scalar.copy(out=h2_fm[d][:, ts(i, 128)], in_=ps)
                            else:
                                nc.vector.tensor_copy(out=h2_fm[d][:, ts(i, 128)],
                                                      in_=ps)

                # ================= Phase 6: FFN1 ============================
                with tc.tile_pool(name="g1P", bufs=1) as g1P:
                    g1 = [g1P.tile([128, TQ], bf16, name=f"g1t{m}")
                          for m in range(FC)]
                    with tc.tile_pool(name="psF", bufs=3, space="PSUM") as psF:
                        for m in range(FC):
                            ps = psF.tile([128, TQ], f32, tag="mm", name="psf")
                            for k in range(DC):
                                nc.tensor.matmul(ps, w1_sb[:, k, ts(m, 128)],
                                                 h2_fm[k][:, 0:TQ],
                                                 start=(k == 0), stop=(k == DC - 1))
                            nc.scalar.activation(out=g1[m], in_=ps, func=Act.Relu,
                                                 bias=b1_sb[:, m:m + 1], scale=1.0)

                    # ================= Phase 7: FFN2 ========================
                    with (
                        tc.tile_pool(name="ph8", bufs=4) as ph8,
                        tc.tile_pool(name="ph8o", bufs=2) as ph8o,
                        tc.tile_pool(name="ps8", bufs=1, space="PSUM") as ps8,
                    ):
                        psum2 = [ps8.tile([128, 512], f32, tag=f"p8_{j}",
                                          name=f"p8_{j}") for j in range(8)]
                        for m in range(FC):
                            w2c = ph8.tile([128, D], bf16, tag="w2c", name="w2c")
                            nc.scalar.dma_start(out=w2c, in_=w2[ts(m, 128), :])
                            for i in range(QS):
                                for n in range(2):
                                    nc.tensor.matmul(
                                        psum2[i * 2 + n],
                                        g1[m][:, ts(i, 128)],
                                        w2c[:, ts(n, 512)],
                                        start=(m == 0), stop=(m == FC - 1))
                        for i in range(QS):
                            ot = ph8o.tile([128, D], f32, tag="ot", name="ot")
                            for n in range(2):
                                nc.vector.tensor_tensor(
                                    out=ot[:, ts(n, 512)],
                                    in0=psum2[i * 2 + n],
                                    in1=x2_sb[i][:, ts(n, 512)], op=Alu.add)
                            if has_b2:
                                nc.vector.tensor_tensor(
                                    out=ot, in0=ot, in1=b2_b, op=Alu.add)
                            nc.sync.dma_start(out=out[ts(i, 128), :], in_=ot)

    nc.compile()
    return nc


def _prep(inputs):
    """Host-side shard prep. Returns in_maps (one dict per core) + flags."""
    x = np.asarray(inputs["x"], np.float32)
    ln1_g = np.asarray(inputs["ln1_g"], np.float32)
    ln1_b = np.asarray(inputs["ln1_b"], np.float32)
    ln2_g = np.asarray(inputs["ln2_g"], np.float32)
    ln2_b = np.asarray(inputs["ln2_b"], np.float32)
    assert np.all(ln1_b == 0.0) and np.all(ln2_b == 0.0), "ln biases must be 0"

    # fold ln gains into the consuming weight matrices
    wq = (ln1_g[:, None] * np.asarray(inputs["Wq"], np.float32)).astype(BF16)
    wk = (ln1_g[:, None] * np.asarray(inputs["Wk"], np.float32)).astype(BF16)
    wv = (ln1_g[:, None] * np.asarray(inputs["Wv"], np.float32)).astype(BF16)
    wo = np.asarray(inputs["Wo"], np.float32).astype(BF16)
    w1 = (ln2_g[:, None] * np.asarray(inputs["W1"], np.float32)).astype(BF16)
    w2 = np.asarray(inputs["W2"], np.float32).astype(BF16)
    b1 = np.ascontiguousarray(np.asarray(inputs["b1"], np.float32))
    bo = np.asarray(inputs["bo"], np.float32)
    b2 = np.asarray(inputs["b2"], np.float32)
    has_bo = bool(np.any(bo != 0.0))
    has_b2 = bool(np.any(b2 != 0.0))

    xb = x.astype(BF16)
    qidx = np.arange(TQ)
    maskD = np.where(qidx[:, None] <= qidx[None, :], np.float32(1.0),
                     np.float32(0.0)).astype(BF16)
    maskD = np.ascontiguousarray(np.concatenate([maskD, maskD], axis=1))

    # Wo rows permuted to the AllGather'd feature order:
    # block j = kq*4 + s holds rank s's pair kq = heads (4s+2kq, 4s+2kq+1)
    # = original Wo rows [s*256 + kq*128 : s*256 + kq*128 + 128).
    wo_p = np.ascontiguousarray(np.concatenate(
        [wo[s * NF + kq * 128: s * NF + kq * 128 + 128]
         for kq in range(2) for s in range(4)]))

    in_maps = []
    for c in range(NCORES):
        g, r = c // 4, c % 4
        m = {
            "x_full": np.ascontiguousarray(xb[g]),
            "x_q": np.ascontiguousarray(x[g, r * TQ:(r + 1) * TQ]),
            "maskD": maskD,
            "wq": np.ascontiguousarray(wq[:, r * NF:(r + 1) * NF]),
            "wk": np.ascontiguousarray(wk[:, r * NF:(r + 1) * NF]),
            "wv": np.ascontiguousarray(wv[:, r * NF:(r + 1) * NF]),
            "wo": wo_p,
            "w1": w1, "w2": w2, "b1": b1,
        }
        if has_bo:
            m["bo"] = bo
        if has_b2:
            m["b2"] = b2
        in_maps.append(m)
    return in_maps, (has_bo, has_b2)


def _run(inputs, profile_dir=None):
    from concourse import bass_utils

    in_maps, flags = _prep(inputs)
    if flags not in _CACHE:
        _CACHE[flags] = _build(flags)
    nc = _CACHE[flags]

    if profile_dir is not None:
        from concourse import bass2jax
        from trn_agent_boot.trn_boot import _ntff_profile_via_ctypes
        hook = _ntff_profile_via_ctypes("/opt/axon/libaxon_pjrt.so")
        with hook(profile_dir, [0]):
            results = bass2jax.run_bass_via_pjrt(nc, in_maps, n_cores=NCORES)
    else:
        res = bass_utils.run_bass_kernel_spmd(
            nc, in_maps, core_ids=list(range(NCORES))
        )
        results = res.results

    out = np.empty((B, T, D), np.float32)
    for c in range(NCORES):
        g, r = c // 4, c % 4
        out[g, r * TQ:(r + 1) * TQ] = results[c]["out"]
    return out


def kernel(**inputs) -> np.ndarray:
    return _run(inputs)

